# revision 45
# baseline (speedup 1.0000x reference)
"""Grouped 3x3 SAME conv on 8 Trainium2 NeuronCores.

Problem: x[16,56,56,256] NHWC, 8 groups of 32->64 channels, 3x3 SAME,
out[16,56,56,512], fp32.

Strategy (hardcoded):
  - Data-parallel over batch: core i handles images [2i, 2i+1].
  - Host-side layout prep: channels-major fp16, zero-padded spatial
    58x58 flattened (+ extra edge columns for tap shifts). No kh
    replication: each group's 32 channels appear exactly once.
  - On device the PE runs in 32x64 tiling mode: 8 independent 32x64
    sub-array tiles, one per group (SBUF strip = g%4 feeding the
    contraction, PSUM half = g//4 receiving the 64 filters). All 8
    groups' matmuls execute concurrently. The 9 conv taps are 9
    PSUM-accumulated matmuls per group whose rhs is the same SBUF
    tile sliced at column offset 58*(dh-1)+(dw-1).
  - PSUM bank k holds groups (k, k+4) stacked [128, 464]; bias is
    added during the PSUM->SBUF copy (DVE/ACT split), which also strips
    the pad columns and casts to fp16. Stores are chunked across three
    DMA queues (SP/ACT HWDGE + gpsimd SWDGE) so they drain during
    compute; input loads are column-chunked so the first spatial tiles
    can start while the rest stream in; dummy matmuls keep the PE warm
    (HAM p-state) during the initial loads.
"""

import numpy as np

G = 8        # groups
P = 32       # in-channels per group
F = 64       # out-channels per group
H = W = 56
HP = WP = 58           # zero-padded spatial
SP = HP * WP           # 3364 padded pixels
SHIFT = WP             # column shift of one image row
P0 = 4                 # extra left pad columns in the device tile
SPP = SP + 8           # device tile width (3372)
N_CORES = 8
B_PER_CORE = 2
NT = 7                 # spatial tiles: 8 image rows each, N=464
TN = 8 * SHIFT         # 464 columns per tile
TV = 8 * W             # 448 valid output columns per tile
NPASS = 9
# tap order: (dh, dw) row-major; shift = 58*(dh-1) + (dw-1)
TAPS = [(dh, dw) for dh in range(3) for dw in range(3)]

_PROG_CACHE = {}


def _build_program():
    import concourse.bacc as bacc
    import concourse.mybir as mybir
    import concourse.tile as tile

    dt = mybir.dt
    nc = bacc.Bacc(
        "TRN2",
        target_bir_lowering=False,
        debug=False,
        num_devices=N_CORES,
    )

    f32 = dt.float32
    f16 = dt.float16
    act_copy = mybir.ActivationFunctionType.Identity

    # inputs: 2 quad tiles per image (groups 0-3 / 4-7), 4 groups x 32ch
    xq = nc.dram_tensor("xq", [B_PER_CORE, 2, 128, SPP], f16,
                        kind="ExternalInput")
    # weights: [32*(g%4)+ch, ((g//4)*9 + pass)*64 + f]
    wq = nc.dram_tensor("wq", [128, 2 * NPASS * F], f16,
                        kind="ExternalInput")
    # bias: [64*h + f, k] for group k + 4*h
    bq = nc.dram_tensor("bq", [128, 4], f32, kind="ExternalInput")
    outT = nc.dram_tensor("outT", [B_PER_CORE, 4, 128, NT * TV], f16,
                          kind="ExternalOutput")

    with tile.TileContext(nc) as tc:
        with (
            tc.tile_pool(name="const", bufs=1) as cpool,
            tc.tile_pool(name="ot", bufs=2) as opool,
            tc.tile_pool(name="ps", bufs=2, space="PSUM") as ppool,
        ):
            # chunk c covers spatial tiles [t0, t1); cols [lo, hi) with halo
            LCH = [(0, 2, 0, 1056), (2, 4, 928, 1984), (4, 7, 1856, SPP)]
            xtc = {}
            load_q = {0: nc.sync, 1: nc.scalar}
            for q in range(2):
                for ci, (t0, t1, lo, hi) in enumerate(LCH):
                    xtc[0, q, ci] = cpool.tile(
                        [128, hi - lo], f16, name=f"xtc0_{q}_{ci}")
            # weights pass-major in two tiles so passes 0-2 unblock early
            WSPLIT = 3 * 2 * F
            wsb1 = cpool.tile([128, WSPLIT], f16)
            wsb2 = cpool.tile([128, 2 * NPASS * F - WSPLIT], f16)
            bsb = cpool.tile([128, 4], f32)

            # per-queue issue order: critical first chunk, then weights,
            # then the rest (each queue processes its DMAs in order)
            for q in range(2):
                lo, hi = LCH[0][2], LCH[0][3]
                load_q[q].dma_start(xtc[0, q, 0][:], xq[0, q, :, lo:hi])
            nc.sync.dma_start(wsb1[:], wq[:, 0:WSPLIT])
            nc.gpsimd.dma_start(wsb2[:], wq[:, WSPLIT:])
            nc.scalar.dma_start(bsb[:], bq[:])
            for ci in range(1, len(LCH)):
                lo, hi = LCH[ci][2], LCH[ci][3]
                for q in range(2):
                    load_q[q].dma_start(xtc[0, q, ci][:],
                                        xq[0, q, :, lo:hi])
            # image 1 loaded whole: one DMA per quad (fewer DMAs/sems)
            xt1 = {}
            for q in range(2):
                xt1[q] = cpool.tile([128, SPP], f16, name=f"xt1_{q}")
                load_q[q].dma_start(xt1[q][:], xq[1, q, :, :])

            # output store chunks (in units of spatial tiles): stores of a
            # finished chunk drain while later tiles compute
            CHUNKS = [(0, 2), (2, 4), (4, 6), (6, 7)]

            # instances in execution order
            insts = [(b, t) for b in range(B_PER_CORE) for t in range(NT)]

            oc = {}
            for b in range(B_PER_CORE):
                for k in range(4):
                    for ci, (c0, c1) in enumerate(CHUNKS):
                        oc[b, k, ci] = opool.tile(
                            [128, (c1 - c0) * TV], f16,
                            tag=f"o{k}_{ci}", name=f"oc{k}_{ci}")

            # keep the PE warm during the initial loads (HAM drops the PE
            # clock after ~3.4us idle): dummy matmuls on a zeroed tile into
            # the first instance's bank; the real pass-0 start=True resets
            # PSUM so these never affect results
            dummy = cpool.tile([32, F], f16)
            nc.vector.memset(dummy[:], 0.0)
            warm_banks = [ppool.tile([128, TN], f32, tag=f"b{k}",
                                     name=f"wbank{k}") for k in range(4)]
            for i in range(42):
                nc.tensor.matmul(
                    warm_banks[0][0:F, 0:F],
                    dummy[:],
                    dummy[:],
                    start=True, stop=True,
                    tile_position=(0, 0),
                )

            store_q = [nc.gpsimd, nc.sync, nc.gpsimd, nc.scalar]
            for idx, (b, t) in enumerate(insts):
                ci = next(i for i, (c0, c1) in enumerate(CHUNKS)
                          if c0 <= t < c1)
                c0, c1 = CHUNKS[ci]
                m0 = P0 + SHIFT + TN * t
                if idx == 0:
                    banks = warm_banks
                else:
                    banks = [ppool.tile([128, TN], f32, tag=f"b{k}",
                                        name=f"bank{k}") for k in range(4)]
                li = next(i for i, (t0, t1, lo, hi) in enumerate(LCH)
                          if t0 <= t < t1)
                lo = LCH[li][2]
                for p, (dh, dw) in enumerate(TAPS):
                    s = m0 + SHIFT * (dh - 1) + (dw - 1)
                    if p < 3:
                        wtile, wc = wsb1, p * 2
                    else:
                        wtile, wc = wsb2, (p - 3) * 2
                    for g in range(G):
                        r, h = g % 4, g // 4
                        if b == 0:
                            rhs = xtc[0, h, li][32 * r:32 * r + 32,
                                                s - lo:s - lo + TN]
                        else:
                            rhs = xt1[h][32 * r:32 * r + 32, s:s + TN]
                        nc.tensor.matmul(
                            banks[r][64 * h:64 * h + 64, :],
                            wtile[32 * r:32 * r + 32,
                                  (wc + h) * F:(wc + h + 1) * F],
                            rhs,
                            start=(p == 0),
                            stop=(p == NPASS - 1),
                            tile_position=(32 * r, 64 * h),
                        )
                for k in range(4):
                    dst = oc[b, k, ci][:, TV * (t - c0):TV * (t - c0 + 1)]
                    dst = dst.rearrange("p (r c) -> p r c", r=8, c=56)
                    src = banks[k][:].rearrange("p (r c) -> p r c",
                                                r=8, c=58)[:, :, 1:57]
                    if k % 2 == 0:
                        nc.vector.tensor_scalar_add(
                            dst, src, bsb[:, k:k + 1])
                    else:
                        nc.scalar.activation(
                            dst, src, act_copy,
                            bias=bsb[:, k:k + 1])
                if t == c1 - 1:
                    fq = store_q if ci < 3 else [nc.sync, nc.scalar,
                                                nc.sync, nc.scalar]
                    for k in range(4):
                        fq[k].dma_start(
                            outT[b, k, :, TV * c0:TV * c1],
                            oc[b, k, ci][:])

    nc.compile()
    return nc


def _get_program():
    if "nc" not in _PROG_CACHE:
        _PROG_CACHE["nc"] = _build_program()
    return _PROG_CACHE["nc"]


def prepare_in_maps(x, kernels, bias):
    x = np.ascontiguousarray(x, dtype=np.float32)
    kernels = np.ascontiguousarray(kernels, dtype=np.float32)
    bias = np.ascontiguousarray(bias, dtype=np.float32)

    nb = x.shape[0]
    # zero-padded channels-major: [b, g, c, SPP] fp16, image at col P0,
    # interior at P0 + (58r + 1 + col offsets)
    xpad = np.zeros((nb, G, P, SPP), np.float16)
    xv = x.transpose(0, 3, 1, 2).reshape(nb, G, P, H, W).astype(np.float16)
    xpad.reshape(nb, G, P, SPP)[:, :, :, :] = 0
    core = xpad[:, :, :, P0:P0 + SP].reshape(nb, G, P, HP, WP)
    core[:, :, :, 1:1 + H, 1:1 + W] = xv
    # quad tiles: [b, q, 128, SPP], group 4q + r at partitions 32r..
    xqv = xpad.reshape(nb, 2, 4 * P, SPP)

    # weights pass-major: wq[32*(g%4)+ch, (p*2 + g//4)*64 + f]
    wq = np.zeros((128, 2 * NPASS * F), np.float16)
    for g in range(G):
        r, h = g % 4, g // 4
        for p, (dh, dw) in enumerate(TAPS):
            wq[32 * r:32 * r + 32, (p * 2 + h) * F:
               (p * 2 + h + 1) * F] = kernels[g, dh, dw]

    # bias: bq[64h+f, k] = bias[(k+4h)*64+f]
    bq = np.empty((128, 4), np.float32)
    for k in range(4):
        for h in range(2):
            bq[64 * h:64 * h + 64, k] = bias[(k + 4 * h) * F:
                                             (k + 4 * h + 1) * F]

    return [
        {"xq": np.ascontiguousarray(
            xqv[i * B_PER_CORE:(i + 1) * B_PER_CORE]),
         "wq": wq, "bq": bq}
        for i in range(N_CORES)
    ]


def gather_output(results, nb):
    out = np.empty((nb, H, W, G * F), np.float32)
    for i in range(N_CORES):
        # o[b, k, 64h+f, 448t + j]; valid interior cols only
        o = results[i]["outT"].astype(np.float32)
        o = o.reshape(B_PER_CORE, 4, 2, F, NT * TV)
        # channels c = (k + 4h)*64 + f -> order (h, k, f)
        o = o.transpose(0, 2, 1, 3, 4).reshape(B_PER_CORE, G * F, H, W)
        out[i * B_PER_CORE:(i + 1) * B_PER_CORE] = o.transpose(0, 2, 3, 1)
    return out


def kernel(x, kernels, bias):
    from concourse.bass_utils import run_bass_kernel_spmd

    nc = _get_program()
    in_maps = prepare_in_maps(x, kernels, bias)
    res = run_bass_kernel_spmd(nc, in_maps, list(range(N_CORES)))
    return gather_output(res.results, np.asarray(x).shape[0])
